# revision 10
# baseline (speedup 1.0000x reference)
"""Cross-attention kernel for TRN2, batch-parallel over 8 NeuronCores.

Problem shapes (hardcoded): B=8, C1=C2=256, H=W=32 (S=1024), NH=8, KD=VD=64.

Per-core program (core b computes batch element b, no collectives):
  X1 = input1[b] as [C1, S1] (natural layout), X2 likewise.
  K1T = Wk1 @ X1   -> [512, S1]   (head h rows h*64:(h+1)*64)   f32r matmul
  K2T = Wk2 @ X2   -> [512, S2]
  V2  = X2.T @ Wv2.T -> [S2, 512] natural layout, stored per-head with a
        ones column appended ([128, 8, 65] per s2-chunk, bf16)
  heads processed in pairs (2c, 2c+1) sharing K-chunk c, software-pipelined:
    step s2: QK matmuls for both heads (row groups 0/64 run concurrently),
             AV matmuls for step s2-1 (gated on exp), exp(scoresT/8) on ACT.
    scoresT layout [s2_blk=128, q=1024] avoids all on-chip transposes; the
    plain exp (no max subtraction) equals softmax exactly since scores are
    O(1).  AV lhsT = [v2|1] so PSUM row 64 accumulates the softmax denom.
  normalize: avs=copy(av_psum); rcp=reciprocal_approx_fast(avs);
             denom row -> DRAM -> partition-broadcast back; oall=avs*rcp_rep
  finalT [C1, S1] = WoT.T @ concat_h(oall_h)  (f32r, K=64 per-head chunks)
  y = finalT reshaped [C1, H, W]  == output[b] layout exactly.
"""

import sys

for _p in ("/opt/trn_rl_repo", "/root/.axon_site/_ro/trn_rl_repo"):
    if _p not in sys.path:
        sys.path.append(_p)

import numpy as np

import concourse.bass as bass
import concourse.mybir as mybir
import concourse.tile as tile
from concourse import bacc, bass_utils

F32 = mybir.dt.float32
F32R = mybir.dt.float32r
BF16 = mybir.dt.bfloat16

B = 8
C1 = 256
S1 = 1024
C2 = 256
S2 = 1024
NH = 8
KD = 64
VD = 64
P = 128


def build_nc(dump=False):
    nc = bacc.Bacc(
        "TRN2",
        target_bir_lowering=False,
        debug=False,
        enable_asserts=False,
        num_devices=B,
    )

    x1 = nc.dram_tensor("x1", [C1, S1], F32R, kind="ExternalInput").ap()
    x2 = nc.dram_tensor("x2", [C2, S2], F32R, kind="ExternalInput").ap()
    wk1t = nc.dram_tensor("wk1t", [C1, NH * KD], F32R, kind="ExternalInput").ap()
    wk2t = nc.dram_tensor("wk2t", [C2, NH * KD], F32R, kind="ExternalInput").ap()
    wv2t = nc.dram_tensor("wv2t", [C2, NH * VD], F32R, kind="ExternalInput").ap()
    wot = nc.dram_tensor("wot", [NH * VD, C1], F32R, kind="ExternalInput").ap()
    y = nc.dram_tensor("y", [C1, S1], F32, kind="ExternalOutput").ap()
    dumps = {}
    if dump:
        for nm, shape in (
            ("d_k1t", [P, S1]),
            ("d_v2a", [P, NH * (VD + 1)]),
            ("d_qk0", [P, S1]),
            ("d_expt", [P, S1]),
            ("d_av", [VD + 1, S1]),
            ("d_recip", [1, S1]),
            ("d_oall", [64, S1]),
        ):
            dumps[nm] = nc.dram_tensor(nm, shape, F32, kind="ExternalOutput").ap()

    with tile.TileContext(nc) as tc:
        with (
            tc.tile_pool(name="const", bufs=1) as cpool,
            tc.tile_pool(name="expt", bufs=7) as epool,
            tc.tile_pool(name="norm", bufs=2) as npool,
            tc.tile_pool(name="yout", bufs=2) as ypool,
            tc.tile_pool(name="pmm", bufs=2, space="PSUM") as pmm,
            tc.tile_pool(name="pav", bufs=2, space="PSUM") as pav,
            tc.tile_pool(name="dscr", bufs=2, space="DRAM") as dpool,
        ):
            # ---- load inputs ----
            x1_sb = [cpool.tile([P, S1], F32R, name=f"x1_{c}") for c in range(2)]
            x2_sb = [cpool.tile([P, S2], F32R, name=f"x2_{c}") for c in range(2)]
            wk1t_sb = [cpool.tile([P, 512], F32R, name=f"wk1t_{c}") for c in range(2)]
            wk2t_sb = [cpool.tile([P, 512], F32R, name=f"wk2t_{c}") for c in range(2)]
            wv2t_sb = [cpool.tile([P, 512], F32R, name=f"wv2t_{c}") for c in range(2)]
            wot_sb = [cpool.tile([64, C1], F32R, name=f"wot_{h}") for h in range(NH)]
            for c in range(2):
                nc.sync.dma_start(x1_sb[c][:], x1[c * P : (c + 1) * P, :])
                nc.sync.dma_start(x2_sb[c][:], x2[c * P : (c + 1) * P, :])
                nc.sync.dma_start(wk1t_sb[c][:], wk1t[c * P : (c + 1) * P, :])
                nc.sync.dma_start(wk2t_sb[c][:], wk2t[c * P : (c + 1) * P, :])
                nc.sync.dma_start(wv2t_sb[c][:], wv2t[c * P : (c + 1) * P, :])
            for h in range(NH):
                nc.sync.dma_start(wot_sb[h][:], wot[h * 64 : (h + 1) * 64, :])

            k1t_sb = [cpool.tile([P, S1], BF16, name=f"k1t_{m}") for m in range(4)]
            k2t_sb = [cpool.tile([P, S2], BF16, name=f"k2t_{m}") for m in range(4)]
            # v2 with per-head ones column: [128, head, 65]
            v2a_sb = [
                cpool.tile([P, NH, VD + 1], BF16, name=f"v2a_{s}") for s in range(8)
            ]
            oall_sb = [cpool.tile([64, S1], F32R, name=f"oall_{h}") for h in range(NH)]

            def emit_proj_chunk(wt_sb, xs_sb, kt, m, dump_to=None):
                """kt[m] (bf16 [128, S]) = (wt chunk).T @ xs, both f32r."""
                ps = pmm.tile([P, 1024], F32, tag="qk", name=f"pj_{kt[m].name}")
                for nh_ in range(2):
                    for k in range(2):
                        nc.tensor.matmul(
                            ps[:, nh_ * 512 : (nh_ + 1) * 512],
                            wt_sb[k][:, m * P : (m + 1) * P],
                            xs_sb[k][:, nh_ * 512 : (nh_ + 1) * 512],
                            start=(k == 0),
                            stop=(k == 1),
                        )
                nc.vector.tensor_copy(out=kt[m][:], in_=ps[:])
                if dump_to is not None:
                    dt_ = ypool.tile([P, S1], F32, tag="dmp", name="dmp_k")
                    nc.vector.tensor_copy(out=dt_[:], in_=kt[m][:])
                    nc.sync.dma_start(dump_to, dt_[:])

            # ---- prologue: K-chunk 0 projections + all of V2 ----
            emit_proj_chunk(
                wk1t_sb, x1_sb, k1t_sb, 0, dumps.get("d_k1t") if dump else None
            )
            emit_proj_chunk(wk2t_sb, x2_sb, k2t_sb, 0)
            for sp in range(4):  # pairs of s2 chunks
                ps = pmm.tile([P, 1024], F32, tag="qk", name=f"pv2_{sp}")
                for half in range(2):
                    s = 2 * sp + half
                    for k in range(2):
                        nc.tensor.matmul(
                            ps[:, half * 512 : (half + 1) * 512],
                            x2_sb[k][:, s * P : (s + 1) * P],
                            wv2t_sb[k][:],
                            start=(k == 0),
                            stop=(k == 1),
                        )
                for half in range(2):
                    s = 2 * sp + half
                    nc.vector.memset(v2a_sb[s][:, :, VD : VD + 1], 1.0)
                    nc.vector.tensor_copy(
                        out=v2a_sb[s][:, :, 0:VD],
                        in_=ps[:, half * 512 : (half + 1) * 512].rearrange(
                            "p (h c) -> p h c", c=VD
                        ),
                    )
                    if dump and s == 0:
                        dt_ = ypool.tile(
                            [P, NH * (VD + 1)], F32, tag="dmp2", name="dv2a"
                        )
                        nc.vector.tensor_copy(
                            out=dt_[:].rearrange("p (h c) -> p h c", c=VD + 1),
                            in_=v2a_sb[0][:],
                        )
                        nc.sync.dma_start(dumps["d_v2a"], dt_[:])

            # ---- attention: head pairs, software pipelined ----
            for c in range(4):
                pair = (2 * c, 2 * c + 1)
                av_ps = {
                    h: pav.tile([VD + 1, S1], F32, tag="av", name=f"av_{h}")
                    for h in pair
                }
                ets = []  # per step: {h: exp tile}

                def emit_av(s2, _av=av_ps, _ets=None, _pair=pair):
                    for h in _pair:
                        et = _ets[s2][h]
                        for nh_ in range(2):
                            nc.tensor.matmul(
                                _av[h][:, nh_ * 512 : (nh_ + 1) * 512],
                                v2a_sb[s2][:, h, :],
                                et[:, nh_ * 512 : (nh_ + 1) * 512],
                                start=(s2 == 0),
                                stop=(s2 == 7),
                                skip_group_check=True,
                            )

                for s2 in range(8):
                    qk = {}
                    for h in pair:
                        qk[h] = pmm.tile([P, S1], F32, tag="qk", name=f"qk_{h}_{s2}")
                    # paired emission: alternate row groups so the two heads'
                    # matmuls run concurrently in the PE array
                    for nh_ in range(2):
                        for h in pair:
                            ro = (h % 2) * 64
                            nc.tensor.matmul(
                                qk[h][:, nh_ * 512 : (nh_ + 1) * 512],
                                k2t_sb[c][ro : ro + 64, s2 * P : (s2 + 1) * P],
                                k1t_sb[c][ro : ro + 64, nh_ * 512 : (nh_ + 1) * 512],
                                start=True,
                                stop=True,
                            )
                    if s2 >= 1:
                        emit_av(s2 - 1, _ets=ets)
                    step = {}
                    for h in pair:
                        et = epool.tile(
                            [P, S1], BF16, tag="expt", name=f"et_{h}_{s2}"
                        )
                        nc.scalar.activation(
                            et[:],
                            qk[h][:],
                            mybir.ActivationFunctionType.Exp,
                            scale=0.125,
                        )
                        step[h] = et
                        if dump and h == 0 and s2 == 0:
                            dt_ = ypool.tile([P, S1], F32, tag="dmp", name="dqk0")
                            nc.vector.tensor_copy(out=dt_[:], in_=qk[h][:])
                            nc.sync.dma_start(dumps["d_qk0"], dt_[:])
                            dt2 = ypool.tile([P, S1], F32, tag="dmp", name="dexpt")
                            nc.vector.tensor_copy(out=dt2[:], in_=et[:])
                            nc.sync.dma_start(dumps["d_expt"], dt2[:])
                    ets.append(step)
                emit_av(7, _ets=ets)

                # next pair's K-chunk projections run on PE while DVE/DMA
                # handle the normalize tails below
                if c < 3:
                    emit_proj_chunk(wk1t_sb, x1_sb, k1t_sb, c + 1)
                    emit_proj_chunk(wk2t_sb, x2_sb, k2t_sb, c + 1)

                # normalize both heads
                for h in pair:
                    avs = npool.tile([VD + 1, S1], F32, tag="avs", name=f"avs_{h}")
                    nc.vector.tensor_copy(out=avs[:], in_=av_ps[h][:])
                    rcp = npool.tile([VD + 1, S1], F32, tag="rcp", name=f"rcp_{h}")
                    nc.vector.reciprocal_approx_fast(rcp[:], avs[:])
                    rdram = dpool.tile([S1], F32, tag="rd", name=f"rd_{h}")
                    nc.sync.dma_start(rdram[:], rcp[VD : VD + 1, :])
                    rep = npool.tile([64, S1], F32, tag="rep", name=f"rep_{h}")
                    nc.sync.dma_start(rep[:], rdram[None, :].to_broadcast((64, S1)))
                    nc.vector.tensor_mul(
                        out=oall_sb[h][:], in0=avs[0:VD, :], in1=rep[:]
                    )
                    if dump and h == 0:
                        dt_ = ypool.tile([VD + 1, S1], F32, tag="dmp", name="dav")
                        nc.vector.tensor_copy(out=dt_[:], in_=avs[:])
                        nc.sync.dma_start(dumps["d_av"], dt_[:])
                        nc.sync.dma_start(dumps["d_recip"], rcp[VD : VD + 1, :])
                        dt2 = ypool.tile([64, S1], F32, tag="dmp", name="doall")
                        nc.vector.tensor_copy(out=dt2[:], in_=oall_sb[0][:])
                        nc.sync.dma_start(dumps["d_oall"], dt2[:])

            # ---- final projection: y[mt] = sum_h WoT_h.T @ oall_h ----
            for mt in range(2):
                fin = pmm.tile([P, S1], F32, tag="qk", name=f"fin_{mt}")
                for nh_ in range(2):
                    for h in range(NH):
                        nc.tensor.matmul(
                            fin[:, nh_ * 512 : (nh_ + 1) * 512],
                            wot_sb[h][:, mt * P : (mt + 1) * P],
                            oall_sb[h][:, nh_ * 512 : (nh_ + 1) * 512],
                            start=(h == 0),
                            stop=(h == NH - 1),
                        )
                ysb = ypool.tile([P, S1], F32, tag="y", name=f"y_{mt}")
                nc.vector.tensor_copy(out=ysb[:], in_=fin[:])
                nc.sync.dma_start(y[mt * P : (mt + 1) * P, :], ysb[:])

    nc.compile()
    return nc


_nc_cache = None


def _get_nc():
    global _nc_cache
    if _nc_cache is None:
        _nc_cache = build_nc()
    return _nc_cache


def _make_in_maps(input1, input2, Wk1, Wk2, Wv2, Wo):
    input1 = np.ascontiguousarray(np.asarray(input1, dtype=np.float32))
    input2 = np.ascontiguousarray(np.asarray(input2, dtype=np.float32))
    wk1t = np.ascontiguousarray(np.asarray(Wk1, dtype=np.float32).T)
    wk2t = np.ascontiguousarray(np.asarray(Wk2, dtype=np.float32).T)
    wv2t = np.ascontiguousarray(np.asarray(Wv2, dtype=np.float32).T)
    wot = np.ascontiguousarray(np.asarray(Wo, dtype=np.float32).T)
    return [
        {
            "x1": np.ascontiguousarray(input1[b].reshape(C1, S1)),
            "x2": np.ascontiguousarray(input2[b].reshape(C2, S2)),
            "wk1t": wk1t,
            "wk2t": wk2t,
            "wv2t": wv2t,
            "wot": wot,
        }
        for b in range(B)
    ]


def _assemble(results):
    out = np.stack([results[b]["y"] for b in range(B)], axis=0)
    return np.ascontiguousarray(out.reshape(B, C1, 32, 32).astype(np.float32))


def kernel(input1, input2, Wk1, Wk2, Wv2, Wo):
    nc = _get_nc()
    in_maps = _make_in_maps(input1, input2, Wk1, Wk2, Wv2, Wo)
    res = bass_utils.run_bass_kernel_spmd(nc, in_maps, core_ids=list(range(B)))
    return _assemble(res.results)


def kernel_traced(input1, input2, Wk1, Wk2, Wv2, Wo):
    """Like kernel() but with NTFF profiling; returns (out, BassKernelResults)."""
    nc = _get_nc()
    in_maps = _make_in_maps(input1, input2, Wk1, Wk2, Wv2, Wo)
    res = bass_utils.run_bass_kernel_spmd(
        nc, in_maps, core_ids=list(range(B)), trace=True
    )
    return _assemble(res.results), res


# revision 11
# speedup vs baseline: 1.3699x; 1.3699x over previous
"""Cross-attention kernel for TRN2, batch-parallel over 8 NeuronCores.

Problem shapes (hardcoded): B=8, C1=C2=256, H=W=32 (S=1024), NH=8, KD=VD=64.

Per-core program (core b computes batch element b, no collectives):
  X1 = input1[b] as [C1, S1] (natural layout), X2 likewise.
  K1T = Wk1 @ X1   -> [512, S1]   (head h rows h*64:(h+1)*64)   f32r matmul
  K2T = Wk2 @ X2   -> [512, S2]
  V2  = X2.T @ Wv2.T -> [S2, 512] natural layout, stored per-head with a
        ones column appended ([128, 8, 65] per s2-chunk, bf16)
  heads processed in pairs (2c, 2c+1) sharing K-chunk c, software-pipelined:
    step s2: QK matmuls for both heads (row groups 0/64 run concurrently),
             AV matmuls for step s2-1 (gated on exp), exp(scoresT/8) on ACT.
    scoresT layout [s2_blk=128, q=1024] avoids all on-chip transposes; the
    plain exp (no max subtraction) equals softmax exactly since scores are
    O(1).  AV lhsT = [v2|1] so PSUM row 64 accumulates the softmax denom.
  normalize: avs=copy(av_psum); rcp=reciprocal_approx_fast(avs);
             denom row -> DRAM -> partition-broadcast back; oall=avs*rcp_rep
  finalT [C1, S1] = WoT.T @ concat_h(oall_h)  (f32r, K=64 per-head chunks)
  y = finalT reshaped [C1, H, W]  == output[b] layout exactly.
"""

import sys

for _p in ("/opt/trn_rl_repo", "/root/.axon_site/_ro/trn_rl_repo"):
    if _p not in sys.path:
        sys.path.append(_p)

import numpy as np

import concourse.bass as bass
import concourse.mybir as mybir
import concourse.tile as tile
from concourse import bacc, bass_utils

F32 = mybir.dt.float32
F32R = mybir.dt.float32r
BF16 = mybir.dt.bfloat16

B = 8
C1 = 256
S1 = 1024
C2 = 256
S2 = 1024
NH = 8
KD = 64
VD = 64
P = 128


def build_nc(dump=False):
    nc = bacc.Bacc(
        "TRN2",
        target_bir_lowering=False,
        debug=False,
        enable_asserts=False,
        num_devices=B,
    )

    x1 = nc.dram_tensor("x1", [C1, S1], F32R, kind="ExternalInput").ap()
    x2 = nc.dram_tensor("x2", [C2, S2], F32R, kind="ExternalInput").ap()
    wk1t = nc.dram_tensor("wk1t", [C1, NH * KD], F32R, kind="ExternalInput").ap()
    wk2t = nc.dram_tensor("wk2t", [C2, NH * KD], F32R, kind="ExternalInput").ap()
    wv2t = nc.dram_tensor("wv2t", [C2, NH * VD], F32R, kind="ExternalInput").ap()
    wot = nc.dram_tensor("wot", [NH * VD, C1], F32R, kind="ExternalInput").ap()
    y = nc.dram_tensor("y", [C1, S1], F32, kind="ExternalOutput").ap()
    dumps = {}
    if dump:
        for nm, shape in (
            ("d_k1t", [P, S1]),
            ("d_v2a", [P, NH * (VD + 1)]),
            ("d_qk0", [P, S1]),
            ("d_expt", [P, S1]),
            ("d_av", [VD + 1, S1]),
            ("d_recip", [1, S1]),
            ("d_oall", [64, S1]),
        ):
            dumps[nm] = nc.dram_tensor(nm, shape, F32, kind="ExternalOutput").ap()

    with tile.TileContext(nc) as tc:
        with (
            tc.tile_pool(name="const", bufs=1) as cpool,
            tc.tile_pool(name="expt", bufs=7) as epool,
            tc.tile_pool(name="norm", bufs=2) as npool,
            tc.tile_pool(name="yout", bufs=2) as ypool,
            tc.tile_pool(name="pmm", bufs=2, space="PSUM") as pmm,
            tc.tile_pool(name="pav", bufs=2, space="PSUM") as pav,
            tc.tile_pool(name="dscr", bufs=2, space="DRAM") as dpool,
        ):
            # ---- load inputs ----
            x1_sb = [cpool.tile([P, S1], F32R, name=f"x1_{c}") for c in range(2)]
            x2_sb = [cpool.tile([P, S2], F32R, name=f"x2_{c}") for c in range(2)]
            wk1t_sb = [cpool.tile([P, 512], F32R, name=f"wk1t_{c}") for c in range(2)]
            wk2t_sb = [cpool.tile([P, 512], F32R, name=f"wk2t_{c}") for c in range(2)]
            wv2t_sb = [cpool.tile([P, 512], F32R, name=f"wv2t_{c}") for c in range(2)]
            wot_sb = [cpool.tile([64, C1], F32R, name=f"wot_{h}") for h in range(NH)]
            for c in range(2):
                nc.sync.dma_start(x1_sb[c][:], x1[c * P : (c + 1) * P, :])
                nc.sync.dma_start(x2_sb[c][:], x2[c * P : (c + 1) * P, :])
                nc.sync.dma_start(wk1t_sb[c][:], wk1t[c * P : (c + 1) * P, :])
                nc.sync.dma_start(wk2t_sb[c][:], wk2t[c * P : (c + 1) * P, :])
                nc.sync.dma_start(wv2t_sb[c][:], wv2t[c * P : (c + 1) * P, :])
            for h in range(NH):
                nc.sync.dma_start(wot_sb[h][:], wot[h * 64 : (h + 1) * 64, :])

            k1t_sb = [cpool.tile([P, S1], BF16, name=f"k1t_{m}") for m in range(4)]
            k2t_sb = [cpool.tile([P, S2], BF16, name=f"k2t_{m}") for m in range(4)]
            # v2 with per-head ones column: [128, head, 65]
            v2a_sb = [
                cpool.tile([P, NH, VD + 1], BF16, name=f"v2a_{s}") for s in range(8)
            ]
            oall_sb = [cpool.tile([64, S1], F32R, name=f"oall_{h}") for h in range(NH)]

            def emit_proj_chunk(wt_sb, xs_sb, kt, m, dump_to=None):
                """kt[m] (bf16 [128, S]) = (wt chunk).T @ xs, both f32r."""
                ps = pmm.tile([P, 1024], F32, tag="qk", name=f"pj_{kt[m].name}")
                for nh_ in range(2):
                    for k in range(2):
                        nc.tensor.matmul(
                            ps[:, nh_ * 512 : (nh_ + 1) * 512],
                            wt_sb[k][:, m * P : (m + 1) * P],
                            xs_sb[k][:, nh_ * 512 : (nh_ + 1) * 512],
                            start=(k == 0),
                            stop=(k == 1),
                        )
                nc.vector.tensor_copy(out=kt[m][:], in_=ps[:])
                if dump_to is not None:
                    dt_ = ypool.tile([P, S1], F32, tag="dmp", name="dmp_k")
                    nc.vector.tensor_copy(out=dt_[:], in_=kt[m][:])
                    nc.sync.dma_start(dump_to, dt_[:])

            # ---- prologue: K-chunk 0 projections + all of V2 ----
            emit_proj_chunk(
                wk1t_sb, x1_sb, k1t_sb, 0, dumps.get("d_k1t") if dump else None
            )
            emit_proj_chunk(wk2t_sb, x2_sb, k2t_sb, 0)
            for sp in range(4):  # pairs of s2 chunks
                ps = pmm.tile([P, 1024], F32, tag="qk", name=f"pv2_{sp}")
                for half in range(2):
                    s = 2 * sp + half
                    for k in range(2):
                        nc.tensor.matmul(
                            ps[:, half * 512 : (half + 1) * 512],
                            x2_sb[k][:, s * P : (s + 1) * P],
                            wv2t_sb[k][:],
                            start=(k == 0),
                            stop=(k == 1),
                        )
                for half in range(2):
                    s = 2 * sp + half
                    nc.vector.memset(v2a_sb[s][:, :, VD : VD + 1], 1.0)
                    nc.vector.tensor_copy(
                        out=v2a_sb[s][:, :, 0:VD],
                        in_=ps[:, half * 512 : (half + 1) * 512].rearrange(
                            "p (h c) -> p h c", c=VD
                        ),
                    )
                    if dump and s == 0:
                        dt_ = ypool.tile(
                            [P, NH * (VD + 1)], F32, tag="dmp2", name="dv2a"
                        )
                        nc.vector.tensor_copy(
                            out=dt_[:].rearrange("p (h c) -> p h c", c=VD + 1),
                            in_=v2a_sb[0][:],
                        )
                        nc.sync.dma_start(dumps["d_v2a"], dt_[:])

            # ---- attention: flat (head, s2) pipeline ----
            # PE runs one step ahead of ACT (2 qk psum slots); AV matmuls lag
            # one step so PE has work while ACT computes the current exp.
            av_tiles = {}
            et_tiles = {}
            prev = None

            def emit_av(h, s2):
                et = et_tiles.pop((h, s2))
                for nh_ in range(2):
                    nc.tensor.matmul(
                        av_tiles[h][:, nh_ * 512 : (nh_ + 1) * 512],
                        v2a_sb[s2][:, h, :],
                        et[:, nh_ * 512 : (nh_ + 1) * 512],
                        start=(s2 == 0),
                        stop=(s2 == 7),
                        skip_group_check=True,
                    )

            def emit_normalize(h):
                avs = npool.tile([VD + 1, S1], F32, tag="avs", name=f"avs_{h}")
                nc.vector.tensor_copy(out=avs[:], in_=av_tiles[h][:])
                rcp = npool.tile([VD + 1, S1], F32, tag="rcp", name=f"rcp_{h}")
                nc.vector.reciprocal_approx_fast(rcp[:], avs[:])
                rdram = dpool.tile([S1], F32, tag="rd", name=f"rd_{h}")
                nc.sync.dma_start(rdram[:], rcp[VD : VD + 1, :])
                rep = npool.tile([64, S1], F32, tag="rep", name=f"rep_{h}")
                nc.sync.dma_start(rep[:], rdram[None, :].to_broadcast((64, S1)))
                nc.vector.tensor_mul(out=oall_sb[h][:], in0=avs[0:VD, :], in1=rep[:])
                if dump and h == 0:
                    dt_ = ypool.tile([VD + 1, S1], F32, tag="dmp", name="dav")
                    nc.vector.tensor_copy(out=dt_[:], in_=avs[:])
                    nc.sync.dma_start(dumps["d_av"], dt_[:])
                    nc.sync.dma_start(dumps["d_recip"], rcp[VD : VD + 1, :])
                    dt2 = ypool.tile([64, S1], F32, tag="dmp", name="doall")
                    nc.vector.tensor_copy(out=dt2[:], in_=oall_sb[0][:])
                    nc.sync.dma_start(dumps["d_oall"], dt2[:])

            for h in range(NH):
                ch = h // 2
                ro = (h % 2) * 64
                for s2 in range(8):
                    if s2 == 0:
                        av_tiles[h] = pav.tile(
                            [VD + 1, S1], F32, tag="av", name=f"av_{h}"
                        )
                    qk = pmm.tile([P, S1], F32, tag="qk", name=f"qk_{h}_{s2}")
                    for nh_ in range(2):
                        nc.tensor.matmul(
                            qk[:, nh_ * 512 : (nh_ + 1) * 512],
                            k2t_sb[ch][ro : ro + 64, s2 * P : (s2 + 1) * P],
                            k1t_sb[ch][ro : ro + 64, nh_ * 512 : (nh_ + 1) * 512],
                            start=True,
                            stop=True,
                        )
                    if prev is not None:
                        emit_av(*prev)
                        if prev[1] == 7:
                            emit_normalize(prev[0])
                    et = epool.tile([P, S1], BF16, tag="expt", name=f"et_{h}_{s2}")
                    nc.scalar.activation(
                        et[:], qk[:], mybir.ActivationFunctionType.Exp, scale=0.125
                    )
                    et_tiles[(h, s2)] = et
                    if dump and h == 0 and s2 == 0:
                        dt_ = ypool.tile([P, S1], F32, tag="dmp", name="dqk0")
                        nc.vector.tensor_copy(out=dt_[:], in_=qk[:])
                        nc.sync.dma_start(dumps["d_qk0"], dt_[:])
                        dt2 = ypool.tile([P, S1], F32, tag="dmp", name="dexpt")
                        nc.vector.tensor_copy(out=dt2[:], in_=et[:])
                        nc.sync.dma_start(dumps["d_expt"], dt2[:])
                    prev = (h, s2)
                    # odd head, step 1: next K-chunk projections, using a psum
                    # slot from the (larger-sized) pav pool just vacated by the
                    # head-(h-1) normalize copy
                    if h % 2 == 1 and s2 == 1 and ch < 3:
                        for wt_sb, xs_sb, kt in (
                            (wk1t_sb, x1_sb, k1t_sb),
                            (wk2t_sb, x2_sb, k2t_sb),
                        ):
                            m = ch + 1
                            ps = pav.tile(
                                [P, 1024], F32, tag="av", name=f"pj2_{kt[m].name}"
                            )
                            for nh_ in range(2):
                                for k in range(2):
                                    nc.tensor.matmul(
                                        ps[:, nh_ * 512 : (nh_ + 1) * 512],
                                        wt_sb[k][:, m * P : (m + 1) * P],
                                        xs_sb[k][:, nh_ * 512 : (nh_ + 1) * 512],
                                        start=(k == 0),
                                        stop=(k == 1),
                                    )
                            nc.vector.tensor_copy(out=kt[m][:], in_=ps[:])
            emit_av(*prev)
            emit_normalize(7)

            # ---- final projection: y[mt] = sum_h WoT_h.T @ oall_h ----
            for mt in range(2):
                fin = pmm.tile([P, S1], F32, tag="qk", name=f"fin_{mt}")
                for nh_ in range(2):
                    for h in range(NH):
                        nc.tensor.matmul(
                            fin[:, nh_ * 512 : (nh_ + 1) * 512],
                            wot_sb[h][:, mt * P : (mt + 1) * P],
                            oall_sb[h][:, nh_ * 512 : (nh_ + 1) * 512],
                            start=(h == 0),
                            stop=(h == NH - 1),
                        )
                ysb = ypool.tile([P, S1], F32, tag="y", name=f"y_{mt}")
                nc.vector.tensor_copy(out=ysb[:], in_=fin[:])
                nc.sync.dma_start(y[mt * P : (mt + 1) * P, :], ysb[:])

    nc.compile()
    return nc


_nc_cache = None


def _get_nc():
    global _nc_cache
    if _nc_cache is None:
        _nc_cache = build_nc()
    return _nc_cache


def _make_in_maps(input1, input2, Wk1, Wk2, Wv2, Wo):
    input1 = np.ascontiguousarray(np.asarray(input1, dtype=np.float32))
    input2 = np.ascontiguousarray(np.asarray(input2, dtype=np.float32))
    wk1t = np.ascontiguousarray(np.asarray(Wk1, dtype=np.float32).T)
    wk2t = np.ascontiguousarray(np.asarray(Wk2, dtype=np.float32).T)
    wv2t = np.ascontiguousarray(np.asarray(Wv2, dtype=np.float32).T)
    wot = np.ascontiguousarray(np.asarray(Wo, dtype=np.float32).T)
    return [
        {
            "x1": np.ascontiguousarray(input1[b].reshape(C1, S1)),
            "x2": np.ascontiguousarray(input2[b].reshape(C2, S2)),
            "wk1t": wk1t,
            "wk2t": wk2t,
            "wv2t": wv2t,
            "wot": wot,
        }
        for b in range(B)
    ]


def _assemble(results):
    out = np.stack([results[b]["y"] for b in range(B)], axis=0)
    return np.ascontiguousarray(out.reshape(B, C1, 32, 32).astype(np.float32))


def kernel(input1, input2, Wk1, Wk2, Wv2, Wo):
    nc = _get_nc()
    in_maps = _make_in_maps(input1, input2, Wk1, Wk2, Wv2, Wo)
    res = bass_utils.run_bass_kernel_spmd(nc, in_maps, core_ids=list(range(B)))
    return _assemble(res.results)


def kernel_traced(input1, input2, Wk1, Wk2, Wv2, Wo):
    """Like kernel() but with NTFF profiling; returns (out, BassKernelResults)."""
    nc = _get_nc()
    in_maps = _make_in_maps(input1, input2, Wk1, Wk2, Wv2, Wo)
    res = bass_utils.run_bass_kernel_spmd(
        nc, in_maps, core_ids=list(range(B)), trace=True
    )
    return _assemble(res.results), res


# revision 13
# speedup vs baseline: 1.5321x; 1.1184x over previous
"""Cross-attention kernel for TRN2, batch-parallel over 8 NeuronCores.

Problem shapes (hardcoded): B=8, C1=C2=256, H=W=32 (S=1024), NH=8, KD=VD=64.

Per-core program (core b computes batch element b, no collectives):
  X1 = input1[b] as [C1, S1] (natural layout), X2 likewise.
  K1T = Wk1 @ X1   -> [512, S1]   (head h rows h*64:(h+1)*64)   f32r matmul
  K2T = Wk2 @ X2   -> [512, S2]
  V2  = X2.T @ Wv2.T -> [S2, 512] natural layout, stored per-head with a
        ones column appended ([128, 8, 65] per s2-chunk, bf16)
  heads processed in pairs (2c, 2c+1) sharing K-chunk c, software-pipelined:
    step s2: QK matmuls for both heads (row groups 0/64 run concurrently),
             AV matmuls for step s2-1 (gated on exp), exp(scoresT/8) on ACT.
    scoresT layout [s2_blk=128, q=1024] avoids all on-chip transposes; the
    plain exp (no max subtraction) equals softmax exactly since scores are
    O(1).  AV lhsT = [v2|1] so PSUM row 64 accumulates the softmax denom.
  normalize: avs=copy(av_psum); rcp=reciprocal_approx_fast(avs);
             denom row -> DRAM -> partition-broadcast back; oall=avs*rcp_rep
  finalT [C1, S1] = WoT.T @ concat_h(oall_h)  (f32r, K=64 per-head chunks)
  y = finalT reshaped [C1, H, W]  == output[b] layout exactly.
"""

import sys

for _p in ("/opt/trn_rl_repo", "/root/.axon_site/_ro/trn_rl_repo"):
    if _p not in sys.path:
        sys.path.append(_p)

import numpy as np

import concourse.bass as bass
import concourse.mybir as mybir
import concourse.tile as tile
from concourse import bacc, bass_utils

F32 = mybir.dt.float32
F32R = mybir.dt.float32r
BF16 = mybir.dt.bfloat16

B = 8
C1 = 256
S1 = 1024
C2 = 256
S2 = 1024
NH = 8
KD = 64
VD = 64
P = 128


def build_nc(dump=False):
    nc = bacc.Bacc(
        "TRN2",
        target_bir_lowering=False,
        debug=False,
        enable_asserts=False,
        num_devices=B,
    )

    x1 = nc.dram_tensor("x1", [C1, S1], F32R, kind="ExternalInput").ap()
    x2 = nc.dram_tensor("x2", [C2, S2], F32R, kind="ExternalInput").ap()
    wk1t = nc.dram_tensor("wk1t", [C1, NH * KD], F32R, kind="ExternalInput").ap()
    wk2t = nc.dram_tensor("wk2t", [C2, NH * KD], F32R, kind="ExternalInput").ap()
    wv2t = nc.dram_tensor("wv2t", [C2, NH * VD], F32R, kind="ExternalInput").ap()
    wot = nc.dram_tensor("wot", [NH * VD, C1], F32R, kind="ExternalInput").ap()
    y = nc.dram_tensor("y", [C1, S1], F32, kind="ExternalOutput").ap()
    dumps = {}
    if dump:
        for nm, shape in (
            ("d_k1t", [P, S1]),
            ("d_v2a", [P, NH * (VD + 1)]),
            ("d_qk0", [P, S1]),
            ("d_expt", [P, S1]),
            ("d_av", [VD + 1, S1]),
            ("d_recip", [1, S1]),
            ("d_oall", [64, S1]),
        ):
            dumps[nm] = nc.dram_tensor(nm, shape, F32, kind="ExternalOutput").ap()

    with tile.TileContext(nc) as tc:
        with (
            tc.tile_pool(name="const", bufs=1) as cpool,
            tc.tile_pool(name="expt", bufs=7) as epool,
            tc.tile_pool(name="norm", bufs=2) as npool,
            tc.tile_pool(name="yout", bufs=2) as ypool,
            tc.tile_pool(name="pmm", bufs=2, space="PSUM") as pmm,
            tc.tile_pool(name="pav", bufs=2, space="PSUM") as pav,
            tc.tile_pool(name="dscr", bufs=2, space="DRAM") as dpool,
        ):
            # ---- load inputs ----
            x1_sb = [cpool.tile([P, S1], F32R, name=f"x1_{c}") for c in range(2)]
            x2_sb = [cpool.tile([P, S2], F32R, name=f"x2_{c}") for c in range(2)]
            wk1t_sb = [cpool.tile([P, 512], F32R, name=f"wk1t_{c}") for c in range(2)]
            wk2t_sb = [cpool.tile([P, 512], F32R, name=f"wk2t_{c}") for c in range(2)]
            wv2t_sb = [cpool.tile([P, 512], F32R, name=f"wv2t_{c}") for c in range(2)]
            wot_sb = [cpool.tile([64, C1], F32R, name=f"wot_{h}") for h in range(NH)]
            for c in range(2):
                nc.sync.dma_start(x1_sb[c][:], x1[c * P : (c + 1) * P, :])
                nc.sync.dma_start(x2_sb[c][:], x2[c * P : (c + 1) * P, :])
                nc.sync.dma_start(wk1t_sb[c][:], wk1t[c * P : (c + 1) * P, :])
                nc.sync.dma_start(wk2t_sb[c][:], wk2t[c * P : (c + 1) * P, :])
                nc.sync.dma_start(wv2t_sb[c][:], wv2t[c * P : (c + 1) * P, :])
            for h in range(NH):
                nc.sync.dma_start(wot_sb[h][:], wot[h * 64 : (h + 1) * 64, :])

            k1t_sb = [cpool.tile([P, S1], BF16, name=f"k1t_{m}") for m in range(4)]
            k2t_sb = [cpool.tile([P, S2], BF16, name=f"k2t_{m}") for m in range(4)]
            # v2 with per-head ones column: [128, head, 65]
            v2a_sb = [
                cpool.tile([P, NH, VD + 1], BF16, name=f"v2a_{s}") for s in range(8)
            ]
            oall_sb = [cpool.tile([64, S1], F32R, name=f"oall_{h}") for h in range(NH)]

            def emit_proj_chunk(pool, wt_sb, xs_sb, kt, m, dump_to=None):
                """kt[m] (bf16 [128, S]) = (wt chunk).T @ xs, both f32r."""
                tag = "qk" if pool is pmm else "pav"
                ps = pool.tile([P, 1024], F32, tag=tag, name=f"pj_{kt[m].name}")
                for nh_ in range(2):
                    for k in range(2):
                        nc.tensor.matmul(
                            ps[:, nh_ * 512 : (nh_ + 1) * 512],
                            wt_sb[k][:, m * P : (m + 1) * P],
                            xs_sb[k][:, nh_ * 512 : (nh_ + 1) * 512],
                            start=(k == 0),
                            stop=(k == 1),
                        )
                nc.vector.tensor_copy(out=kt[m][:], in_=ps[:])
                if dump_to is not None:
                    dt_ = ypool.tile([P, S1], F32, tag="dmp", name="dmp_k")
                    nc.vector.tensor_copy(out=dt_[:], in_=kt[m][:])
                    nc.sync.dma_start(dump_to, dt_[:])

            def emit_v2_pair(sp):
                ps = pav.tile([P, 1024], F32, tag="pav", name=f"pv2_{sp}")
                for half in range(2):
                    s = 2 * sp + half
                    for k in range(2):
                        nc.tensor.matmul(
                            ps[:, half * 512 : (half + 1) * 512],
                            x2_sb[k][:, s * P : (s + 1) * P],
                            wv2t_sb[k][:],
                            start=(k == 0),
                            stop=(k == 1),
                        )
                for half in range(2):
                    s = 2 * sp + half
                    nc.vector.memset(v2a_sb[s][:, :, VD : VD + 1], 1.0)
                    nc.vector.tensor_copy(
                        out=v2a_sb[s][:, :, 0:VD],
                        in_=ps[:, half * 512 : (half + 1) * 512].rearrange(
                            "p (h c) -> p h c", c=VD
                        ),
                    )
                    if dump and s == 0:
                        dt_ = ypool.tile(
                            [P, NH * (VD + 1)], F32, tag="dmp2", name="dv2a"
                        )
                        nc.vector.tensor_copy(
                            out=dt_[:].rearrange("p (h c) -> p h c", c=VD + 1),
                            in_=v2a_sb[0][:],
                        )
                        nc.sync.dma_start(dumps["d_v2a"], dt_[:])

            # ---- prologue: K-chunk 0 projections (attention gates on these) ----
            emit_proj_chunk(
                pmm, wk1t_sb, x1_sb, k1t_sb, 0, dumps.get("d_k1t") if dump else None
            )
            emit_proj_chunk(pmm, wk2t_sb, x2_sb, k2t_sb, 0)

            # ---- attention: pair-packed (a|b) flat pipeline ----
            # qk psum tile [128, 1024] holds head a's q-half in cols 0:512 and
            # head b's in cols 512:1024 (concurrent row-group matmuls); one exp
            # per tile covers both heads.  AV lags one step so PE always has
            # ready work while ACT streams exps.  V2 and the next K-chunk
            # projections ride the pav pool rotation.
            av_tiles = {}
            et_tiles = {}
            pending = []

            def emit_av(c, s2):
                a, b = 2 * c, 2 * c + 1
                if s2 == 0:
                    for h in (a, b):
                        av_tiles[h] = pav.tile(
                            [VD + 1, S1], F32, tag="pav", name=f"av_{h}"
                        )
                for nh_ in range(2):
                    et = et_tiles[(c, s2, nh_)]
                    for idx, h in enumerate((a, b)):
                        nc.tensor.matmul(
                            av_tiles[h][:, nh_ * 512 : (nh_ + 1) * 512],
                            v2a_sb[s2][:, h, :],
                            et[:, idx * 512 : (idx + 1) * 512],
                            start=(s2 == 0),
                            stop=(s2 == 7),
                            skip_group_check=True,
                        )
                for nh_ in range(2):
                    del et_tiles[(c, s2, nh_)]

            def emit_normalize(h):
                avs = npool.tile([VD + 1, S1], F32, tag="avs", name=f"avs_{h}")
                nc.vector.tensor_copy(out=avs[:], in_=av_tiles[h][:])
                rcp = npool.tile([VD + 1, S1], F32, tag="rcp", name=f"rcp_{h}")
                nc.vector.reciprocal_approx_fast(rcp[:], avs[:])
                rdram = dpool.tile([S1], F32, tag="rd", name=f"rd_{h}")
                nc.gpsimd.dma_start(rdram[:], rcp[VD : VD + 1, :])
                rep = npool.tile([64, S1], F32, tag="rep", name=f"rep_{h}")
                nc.gpsimd.dma_start(rep[:], rdram[None, :].to_broadcast((64, S1)))
                nc.vector.tensor_mul(out=oall_sb[h][:], in0=avs[0:VD, :], in1=rep[:])
                if dump and h == 0:
                    dt_ = ypool.tile([VD + 1, S1], F32, tag="dmp", name="dav")
                    nc.vector.tensor_copy(out=dt_[:], in_=avs[:])
                    nc.sync.dma_start(dumps["d_av"], dt_[:])
                    nc.sync.dma_start(dumps["d_recip"], rcp[VD : VD + 1, :])
                    dt2 = ypool.tile([64, S1], F32, tag="dmp", name="doall")
                    nc.vector.tensor_copy(out=dt2[:], in_=oall_sb[0][:])
                    nc.sync.dma_start(dumps["d_oall"], dt2[:])

            def flush_av(upto):
                while len(pending) > upto:
                    cc, ss = pending.pop(0)
                    emit_av(cc, ss)
                    if ss == 7:
                        emit_normalize(2 * cc)
                        emit_normalize(2 * cc + 1)
                        if cc + 2 <= 3:
                            emit_proj_chunk(pav, wk1t_sb, x1_sb, k1t_sb, cc + 2)
                            emit_proj_chunk(pav, wk2t_sb, x2_sb, k2t_sb, cc + 2)

            for c in range(4):
                a, b = 2 * c, 2 * c + 1
                for s2 in range(8):
                    qks = []
                    for nh_ in range(2):
                        qk = pmm.tile(
                            [P, S1], F32, tag="qk", name=f"qk_{c}_{s2}_{nh_}"
                        )
                        for idx, h in enumerate((a, b)):
                            ro = (h % 2) * 64
                            nc.tensor.matmul(
                                qk[:, idx * 512 : (idx + 1) * 512],
                                k2t_sb[c][ro : ro + 64, s2 * P : (s2 + 1) * P],
                                k1t_sb[c][ro : ro + 64, nh_ * 512 : (nh_ + 1) * 512],
                                start=True,
                                stop=True,
                            )
                        qks.append(qk)
                    if c == 0:
                        if s2 == 0:
                            emit_v2_pair(0)
                            emit_v2_pair(1)
                        elif s2 == 1:
                            emit_v2_pair(2)
                            emit_v2_pair(3)
                        elif s2 == 2:
                            emit_proj_chunk(pav, wk1t_sb, x1_sb, k1t_sb, 1)
                            emit_proj_chunk(pav, wk2t_sb, x2_sb, k2t_sb, 1)
                    flush_av(2 if c == 0 else 1)
                    for nh_ in range(2):
                        et = epool.tile(
                            [P, S1], BF16, tag="expt", name=f"et_{c}_{s2}_{nh_}"
                        )
                        nc.scalar.activation(
                            et[:],
                            qks[nh_][:],
                            mybir.ActivationFunctionType.Exp,
                            scale=0.125,
                        )
                        et_tiles[(c, s2, nh_)] = et
                        if dump and c == 0 and s2 == 0 and nh_ == 0:
                            dt_ = ypool.tile([P, S1], F32, tag="dmp", name="dqk0")
                            nc.vector.tensor_copy(out=dt_[:], in_=qks[0][:])
                            nc.sync.dma_start(dumps["d_qk0"], dt_[:])
                            dt2 = ypool.tile([P, S1], F32, tag="dmp", name="dexpt")
                            nc.vector.tensor_copy(out=dt2[:], in_=et[:])
                            nc.sync.dma_start(dumps["d_expt"], dt2[:])
                    pending.append((c, s2))
            flush_av(0)

            # ---- final projection: y[mt] = sum_h WoT_h.T @ oall_h ----
            for mt in range(2):
                fin = pmm.tile([P, S1], F32, tag="qk", name=f"fin_{mt}")
                for nh_ in range(2):
                    for h in range(NH):
                        nc.tensor.matmul(
                            fin[:, nh_ * 512 : (nh_ + 1) * 512],
                            wot_sb[h][:, mt * P : (mt + 1) * P],
                            oall_sb[h][:, nh_ * 512 : (nh_ + 1) * 512],
                            start=(h == 0),
                            stop=(h == NH - 1),
                        )
                ysb = ypool.tile([P, S1], F32, tag="y", name=f"y_{mt}")
                nc.vector.tensor_copy(out=ysb[:], in_=fin[:])
                nc.sync.dma_start(y[mt * P : (mt + 1) * P, :], ysb[:])

    nc.compile()
    return nc


_nc_cache = None


def _get_nc():
    global _nc_cache
    if _nc_cache is None:
        _nc_cache = build_nc()
    return _nc_cache


def _make_in_maps(input1, input2, Wk1, Wk2, Wv2, Wo):
    input1 = np.ascontiguousarray(np.asarray(input1, dtype=np.float32))
    input2 = np.ascontiguousarray(np.asarray(input2, dtype=np.float32))
    wk1t = np.ascontiguousarray(np.asarray(Wk1, dtype=np.float32).T)
    wk2t = np.ascontiguousarray(np.asarray(Wk2, dtype=np.float32).T)
    wv2t = np.ascontiguousarray(np.asarray(Wv2, dtype=np.float32).T)
    wot = np.ascontiguousarray(np.asarray(Wo, dtype=np.float32).T)
    return [
        {
            "x1": np.ascontiguousarray(input1[b].reshape(C1, S1)),
            "x2": np.ascontiguousarray(input2[b].reshape(C2, S2)),
            "wk1t": wk1t,
            "wk2t": wk2t,
            "wv2t": wv2t,
            "wot": wot,
        }
        for b in range(B)
    ]


def _assemble(results):
    out = np.stack([results[b]["y"] for b in range(B)], axis=0)
    return np.ascontiguousarray(out.reshape(B, C1, 32, 32).astype(np.float32))


def kernel(input1, input2, Wk1, Wk2, Wv2, Wo):
    nc = _get_nc()
    in_maps = _make_in_maps(input1, input2, Wk1, Wk2, Wv2, Wo)
    res = bass_utils.run_bass_kernel_spmd(nc, in_maps, core_ids=list(range(B)))
    return _assemble(res.results)


def kernel_traced(input1, input2, Wk1, Wk2, Wv2, Wo):
    """Like kernel() but with NTFF profiling; returns (out, BassKernelResults)."""
    nc = _get_nc()
    in_maps = _make_in_maps(input1, input2, Wk1, Wk2, Wv2, Wo)
    res = bass_utils.run_bass_kernel_spmd(
        nc, in_maps, core_ids=list(range(B)), trace=True
    )
    return _assemble(res.results), res


# revision 14
# speedup vs baseline: 1.5680x; 1.0234x over previous
"""Cross-attention kernel for TRN2, batch-parallel over 8 NeuronCores.

Problem shapes (hardcoded): B=8, C1=C2=256, H=W=32 (S=1024), NH=8, KD=VD=64.

Per-core program (core b computes batch element b, no collectives):
  X1 = input1[b] as [C1, S1] (natural layout), X2 likewise.
  K1T = Wk1 @ X1   -> [512, S1]   (head h rows h*64:(h+1)*64)   f32r matmul
  K2T = Wk2 @ X2   -> [512, S2]
  V2  = X2.T @ Wv2.T -> [S2, 512] natural layout, stored per-head with a
        ones column appended ([128, 8, 65] per s2-chunk, bf16)
  heads processed in pairs (2c, 2c+1) sharing K-chunk c, software-pipelined:
    step s2: QK matmuls for both heads (row groups 0/64 run concurrently),
             AV matmuls for step s2-1 (gated on exp), exp(scoresT/8) on ACT.
    scoresT layout [s2_blk=128, q=1024] avoids all on-chip transposes; the
    plain exp (no max subtraction) equals softmax exactly since scores are
    O(1).  AV lhsT = [v2|1] so PSUM row 64 accumulates the softmax denom.
  normalize: avs=copy(av_psum); rcp=reciprocal_approx_fast(avs);
             denom row -> DRAM -> partition-broadcast back; oall=avs*rcp_rep
  finalT [C1, S1] = WoT.T @ concat_h(oall_h)  (f32r, K=64 per-head chunks)
  y = finalT reshaped [C1, H, W]  == output[b] layout exactly.
"""

import sys

for _p in ("/opt/trn_rl_repo", "/root/.axon_site/_ro/trn_rl_repo"):
    if _p not in sys.path:
        sys.path.append(_p)

import numpy as np

import concourse.bass as bass
import concourse.mybir as mybir
import concourse.tile as tile
from concourse import bacc, bass_utils

F32 = mybir.dt.float32
F32R = mybir.dt.float32r
BF16 = mybir.dt.bfloat16

B = 8
C1 = 256
S1 = 1024
C2 = 256
S2 = 1024
NH = 8
KD = 64
VD = 64
P = 128


def build_nc(dump=False):
    nc = bacc.Bacc(
        "TRN2",
        target_bir_lowering=False,
        debug=False,
        enable_asserts=False,
        num_devices=B,
    )

    x1 = nc.dram_tensor("x1", [C1, S1], F32R, kind="ExternalInput").ap()
    x2 = nc.dram_tensor("x2", [C2, S2], F32R, kind="ExternalInput").ap()
    wk1t = nc.dram_tensor("wk1t", [C1, NH * KD], F32R, kind="ExternalInput").ap()
    wk2t = nc.dram_tensor("wk2t", [C2, NH * KD], F32R, kind="ExternalInput").ap()
    wv2t = nc.dram_tensor("wv2t", [C2, NH * VD], F32R, kind="ExternalInput").ap()
    wot = nc.dram_tensor("wot", [NH * VD, C1], F32R, kind="ExternalInput").ap()
    y = nc.dram_tensor("y", [C1, S1], F32, kind="ExternalOutput").ap()
    dumps = {}
    if dump:
        for nm, shape in (
            ("d_k1t", [P, S1]),
            ("d_v2a", [P, NH * (VD + 1)]),
            ("d_qk0", [P, S1]),
            ("d_expt", [P, S1]),
            ("d_av", [VD + 1, S1]),
            ("d_recip", [1, S1]),
            ("d_oall", [64, S1]),
        ):
            dumps[nm] = nc.dram_tensor(nm, shape, F32, kind="ExternalOutput").ap()

    with tile.TileContext(nc) as tc:
        with (
            tc.tile_pool(name="const", bufs=1) as cpool,
            tc.tile_pool(name="expt", bufs=7) as epool,
            tc.tile_pool(name="norm", bufs=2) as npool,
            tc.tile_pool(name="yout", bufs=2) as ypool,
            tc.tile_pool(name="pmm", bufs=2, space="PSUM") as pmm,
            tc.tile_pool(name="pav", bufs=2, space="PSUM") as pav,
            tc.tile_pool(name="dscr", bufs=2, space="DRAM") as dpool,
        ):
            # ---- load inputs ----
            x1_sb = [cpool.tile([P, S1], F32R, name=f"x1_{c}") for c in range(2)]
            x2_sb = [cpool.tile([P, S2], F32R, name=f"x2_{c}") for c in range(2)]
            wk1t_sb = [cpool.tile([P, 512], F32R, name=f"wk1t_{c}") for c in range(2)]
            wk2t_sb = [cpool.tile([P, 512], F32R, name=f"wk2t_{c}") for c in range(2)]
            wv2t_sb = [cpool.tile([P, 512], F32R, name=f"wv2t_{c}") for c in range(2)]
            wot_sb = [cpool.tile([64, C1], F32R, name=f"wot_{h}") for h in range(NH)]
            for c in range(2):
                nc.sync.dma_start(wk1t_sb[c][:], wk1t[c * P : (c + 1) * P, :])
                nc.sync.dma_start(x1_sb[c][:], x1[c * P : (c + 1) * P, :])
            for c in range(2):
                nc.sync.dma_start(wk2t_sb[c][:], wk2t[c * P : (c + 1) * P, :])
                nc.sync.dma_start(x2_sb[c][:], x2[c * P : (c + 1) * P, :])
            for c in range(2):
                nc.sync.dma_start(wv2t_sb[c][:], wv2t[c * P : (c + 1) * P, :])
            for h in range(NH):
                nc.sync.dma_start(wot_sb[h][:], wot[h * 64 : (h + 1) * 64, :])

            k1t_sb = [cpool.tile([P, S1], BF16, name=f"k1t_{m}") for m in range(4)]
            k2t_sb = [cpool.tile([P, S2], BF16, name=f"k2t_{m}") for m in range(4)]
            # v2 with per-head ones column: [128, head, 65]
            v2a_sb = [
                cpool.tile([P, NH, VD + 1], BF16, name=f"v2a_{s}") for s in range(8)
            ]
            oall_sb = [cpool.tile([64, S1], F32R, name=f"oall_{h}") for h in range(NH)]

            def emit_proj_chunk(pool, wt_sb, xs_sb, kt, m, dump_to=None):
                """kt[m] (bf16 [128, S]) = (wt chunk).T @ xs, both f32r."""
                tag = "qk" if pool is pmm else "pav"
                ps = pool.tile([P, 1024], F32, tag=tag, name=f"pj_{kt[m].name}")
                for nh_ in range(2):
                    for k in range(2):
                        nc.tensor.matmul(
                            ps[:, nh_ * 512 : (nh_ + 1) * 512],
                            wt_sb[k][:, m * P : (m + 1) * P],
                            xs_sb[k][:, nh_ * 512 : (nh_ + 1) * 512],
                            start=(k == 0),
                            stop=(k == 1),
                        )
                nc.vector.tensor_copy(out=kt[m][:], in_=ps[:])
                if dump_to is not None:
                    dt_ = ypool.tile([P, S1], F32, tag="dmp", name="dmp_k")
                    nc.vector.tensor_copy(out=dt_[:], in_=kt[m][:])
                    nc.sync.dma_start(dump_to, dt_[:])

            def emit_v2_pair(sp):
                ps = pav.tile([P, 1024], F32, tag="pav", name=f"pv2_{sp}")
                for half in range(2):
                    s = 2 * sp + half
                    for k in range(2):
                        nc.tensor.matmul(
                            ps[:, half * 512 : (half + 1) * 512],
                            x2_sb[k][:, s * P : (s + 1) * P],
                            wv2t_sb[k][:],
                            start=(k == 0),
                            stop=(k == 1),
                        )
                for half in range(2):
                    s = 2 * sp + half
                    nc.vector.memset(v2a_sb[s][:, :, VD : VD + 1], 1.0)
                    nc.vector.tensor_copy(
                        out=v2a_sb[s][:, :, 0:VD],
                        in_=ps[:, half * 512 : (half + 1) * 512].rearrange(
                            "p (h c) -> p h c", c=VD
                        ),
                    )
                    if dump and s == 0:
                        dt_ = ypool.tile(
                            [P, NH * (VD + 1)], F32, tag="dmp2", name="dv2a"
                        )
                        nc.vector.tensor_copy(
                            out=dt_[:].rearrange("p (h c) -> p h c", c=VD + 1),
                            in_=v2a_sb[0][:],
                        )
                        nc.sync.dma_start(dumps["d_v2a"], dt_[:])

            # ---- prologue: K-chunk 0 projections (attention gates on these) ----
            emit_proj_chunk(
                pmm, wk1t_sb, x1_sb, k1t_sb, 0, dumps.get("d_k1t") if dump else None
            )
            emit_proj_chunk(pmm, wk2t_sb, x2_sb, k2t_sb, 0)

            # ---- attention: pair-packed (a|b) flat pipeline ----
            # qk psum tile [128, 1024] holds head a's q-half in cols 0:512 and
            # head b's in cols 512:1024 (concurrent row-group matmuls); one exp
            # per tile covers both heads.  AV lags one step so PE always has
            # ready work while ACT streams exps.  V2 and the next K-chunk
            # projections ride the pav pool rotation.
            av_tiles = {}
            et_tiles = {}
            pending = []

            def emit_av(c, s2):
                a, b = 2 * c, 2 * c + 1
                if s2 == 0:
                    for h in (a, b):
                        av_tiles[h] = pav.tile(
                            [VD + 1, S1], F32, tag="pav", name=f"av_{h}"
                        )
                for nh_ in range(2):
                    et = et_tiles[(c, s2, nh_)]
                    for idx, h in enumerate((a, b)):
                        nc.tensor.matmul(
                            av_tiles[h][:, nh_ * 512 : (nh_ + 1) * 512],
                            v2a_sb[s2][:, h, :],
                            et[:, idx * 512 : (idx + 1) * 512],
                            start=(s2 == 0),
                            stop=(s2 == 7),
                            skip_group_check=True,
                        )
                for nh_ in range(2):
                    del et_tiles[(c, s2, nh_)]

            def emit_normalize(h):
                avs = npool.tile([VD + 1, S1], F32, tag="avs", name=f"avs_{h}")
                if h >= 6:
                    nc.scalar.copy(out=avs[:], in_=av_tiles[h][:])
                else:
                    nc.vector.tensor_copy(out=avs[:], in_=av_tiles[h][:])
                rcp = npool.tile([VD + 1, S1], F32, tag="rcp", name=f"rcp_{h}")
                nc.vector.reciprocal_approx_fast(rcp[:], avs[:])
                rdram = dpool.tile([S1], F32, tag="rd", name=f"rd_{h}")
                nc.gpsimd.dma_start(rdram[:], rcp[VD : VD + 1, :])
                rep = npool.tile([64, S1], F32, tag="rep", name=f"rep_{h}")
                nc.gpsimd.dma_start(rep[:], rdram[None, :].to_broadcast((64, S1)))
                nc.gpsimd.tensor_mul(out=oall_sb[h][:], in0=avs[0:VD, :], in1=rep[:])
                if dump and h == 0:
                    dt_ = ypool.tile([VD + 1, S1], F32, tag="dmp", name="dav")
                    nc.vector.tensor_copy(out=dt_[:], in_=avs[:])
                    nc.sync.dma_start(dumps["d_av"], dt_[:])
                    nc.sync.dma_start(dumps["d_recip"], rcp[VD : VD + 1, :])
                    dt2 = ypool.tile([64, S1], F32, tag="dmp", name="doall")
                    nc.vector.tensor_copy(out=dt2[:], in_=oall_sb[0][:])
                    nc.sync.dma_start(dumps["d_oall"], dt2[:])

            def flush_av(upto):
                while len(pending) > upto:
                    cc, ss = pending.pop(0)
                    emit_av(cc, ss)
                    if ss == 7:
                        emit_normalize(2 * cc)
                        emit_normalize(2 * cc + 1)
                        if cc + 2 <= 3:
                            emit_proj_chunk(pav, wk1t_sb, x1_sb, k1t_sb, cc + 2)
                            emit_proj_chunk(pav, wk2t_sb, x2_sb, k2t_sb, cc + 2)

            for c in range(4):
                a, b = 2 * c, 2 * c + 1
                for s2 in range(8):
                    qks = []
                    for nh_ in range(2):
                        qk = pmm.tile(
                            [P, S1], F32, tag="qk", name=f"qk_{c}_{s2}_{nh_}"
                        )
                        for idx, h in enumerate((a, b)):
                            ro = (h % 2) * 64
                            nc.tensor.matmul(
                                qk[:, idx * 512 : (idx + 1) * 512],
                                k2t_sb[c][ro : ro + 64, s2 * P : (s2 + 1) * P],
                                k1t_sb[c][ro : ro + 64, nh_ * 512 : (nh_ + 1) * 512],
                                start=True,
                                stop=True,
                            )
                        qks.append(qk)
                    if c == 0:
                        if s2 == 0:
                            emit_v2_pair(0)
                            emit_v2_pair(1)
                        elif s2 == 1:
                            emit_v2_pair(2)
                            emit_v2_pair(3)
                        elif s2 == 2:
                            emit_proj_chunk(pav, wk1t_sb, x1_sb, k1t_sb, 1)
                            emit_proj_chunk(pav, wk2t_sb, x2_sb, k2t_sb, 1)
                    flush_av(2 if c == 0 else 1)
                    for nh_ in range(2):
                        et = epool.tile(
                            [P, S1], BF16, tag="expt", name=f"et_{c}_{s2}_{nh_}"
                        )
                        nc.scalar.activation(
                            et[:],
                            qks[nh_][:],
                            mybir.ActivationFunctionType.Exp,
                            scale=0.125,
                        )
                        et_tiles[(c, s2, nh_)] = et
                        if dump and c == 0 and s2 == 0 and nh_ == 0:
                            dt_ = ypool.tile([P, S1], F32, tag="dmp", name="dqk0")
                            nc.vector.tensor_copy(out=dt_[:], in_=qks[0][:])
                            nc.sync.dma_start(dumps["d_qk0"], dt_[:])
                            dt2 = ypool.tile([P, S1], F32, tag="dmp", name="dexpt")
                            nc.vector.tensor_copy(out=dt2[:], in_=et[:])
                            nc.sync.dma_start(dumps["d_expt"], dt2[:])
                    pending.append((c, s2))
            flush_av(0)

            # ---- final projection: y[mt] = sum_h WoT_h.T @ oall_h ----
            fins = [
                pmm.tile([P, S1], F32, tag="qk", name=f"fin_{mt}") for mt in range(2)
            ]
            for h in range(NH):
                for mt in range(2):
                    for nh_ in range(2):
                        nc.tensor.matmul(
                            fins[mt][:, nh_ * 512 : (nh_ + 1) * 512],
                            wot_sb[h][:, mt * P : (mt + 1) * P],
                            oall_sb[h][:, nh_ * 512 : (nh_ + 1) * 512],
                            start=(h == 0),
                            stop=(h == NH - 1),
                            skip_group_check=True,
                        )
            for mt in range(2):
                for half in range(2):
                    ysb = ypool.tile([P, 512], F32, tag="y", name=f"y_{mt}_{half}")
                    nc.scalar.copy(
                        out=ysb[:], in_=fins[mt][:, half * 512 : (half + 1) * 512]
                    )
                    nc.sync.dma_start(
                        y[mt * P : (mt + 1) * P, half * 512 : (half + 1) * 512],
                        ysb[:],
                    )

    nc.compile()
    return nc


_nc_cache = None


def _get_nc():
    global _nc_cache
    if _nc_cache is None:
        _nc_cache = build_nc()
    return _nc_cache


def _make_in_maps(input1, input2, Wk1, Wk2, Wv2, Wo):
    input1 = np.ascontiguousarray(np.asarray(input1, dtype=np.float32))
    input2 = np.ascontiguousarray(np.asarray(input2, dtype=np.float32))
    wk1t = np.ascontiguousarray(np.asarray(Wk1, dtype=np.float32).T)
    wk2t = np.ascontiguousarray(np.asarray(Wk2, dtype=np.float32).T)
    wv2t = np.ascontiguousarray(np.asarray(Wv2, dtype=np.float32).T)
    wot = np.ascontiguousarray(np.asarray(Wo, dtype=np.float32).T)
    return [
        {
            "x1": np.ascontiguousarray(input1[b].reshape(C1, S1)),
            "x2": np.ascontiguousarray(input2[b].reshape(C2, S2)),
            "wk1t": wk1t,
            "wk2t": wk2t,
            "wv2t": wv2t,
            "wot": wot,
        }
        for b in range(B)
    ]


def _assemble(results):
    out = np.stack([results[b]["y"] for b in range(B)], axis=0)
    return np.ascontiguousarray(out.reshape(B, C1, 32, 32).astype(np.float32))


def kernel(input1, input2, Wk1, Wk2, Wv2, Wo):
    nc = _get_nc()
    in_maps = _make_in_maps(input1, input2, Wk1, Wk2, Wv2, Wo)
    res = bass_utils.run_bass_kernel_spmd(nc, in_maps, core_ids=list(range(B)))
    return _assemble(res.results)


def kernel_traced(input1, input2, Wk1, Wk2, Wv2, Wo):
    """Like kernel() but with NTFF profiling; returns (out, BassKernelResults)."""
    nc = _get_nc()
    in_maps = _make_in_maps(input1, input2, Wk1, Wk2, Wv2, Wo)
    res = bass_utils.run_bass_kernel_spmd(
        nc, in_maps, core_ids=list(range(B)), trace=True
    )
    return _assemble(res.results), res


# revision 15
# speedup vs baseline: 1.6414x; 1.0468x over previous
"""Cross-attention kernel for TRN2, batch-parallel over 8 NeuronCores.

Problem shapes (hardcoded): B=8, C1=C2=256, H=W=32 (S=1024), NH=8, KD=VD=64.

Per-core program (core b computes batch element b, no collectives):
  X1 = input1[b] as [C1, S1] (natural layout), X2 likewise.
  K1T = Wk1 @ X1   -> [512, S1]   (head h rows h*64:(h+1)*64)   f32r matmul
  K2T = Wk2 @ X2   -> [512, S2]
  V2  = X2.T @ Wv2.T -> [S2, 512] natural layout, stored per-head with a
        ones column appended ([128, 8, 65] per s2-chunk, bf16)
  heads processed in pairs (2c, 2c+1) sharing K-chunk c, software-pipelined:
    step s2: QK matmuls for both heads (row groups 0/64 run concurrently),
             AV matmuls for step s2-1 (gated on exp), exp(scoresT/8) on ACT.
    scoresT layout [s2_blk=128, q=1024] avoids all on-chip transposes; the
    plain exp (no max subtraction) equals softmax exactly since scores are
    O(1).  AV lhsT = [v2|1] so PSUM row 64 accumulates the softmax denom.
  normalize: avs=copy(av_psum); rcp=reciprocal_approx_fast(avs);
             denom row -> DRAM -> partition-broadcast back; oall=avs*rcp_rep
  finalT [C1, S1] = WoT.T @ concat_h(oall_h)  (f32r, K=64 per-head chunks)
  y = finalT reshaped [C1, H, W]  == output[b] layout exactly.
"""

import sys

for _p in ("/opt/trn_rl_repo", "/root/.axon_site/_ro/trn_rl_repo"):
    if _p not in sys.path:
        sys.path.append(_p)

import numpy as np

import concourse.bass as bass
import concourse.mybir as mybir
import concourse.tile as tile
from concourse import bacc, bass_utils

F32 = mybir.dt.float32
F32R = mybir.dt.float32r
BF16 = mybir.dt.bfloat16

B = 8
C1 = 256
S1 = 1024
C2 = 256
S2 = 1024
NH = 8
KD = 64
VD = 64
P = 128


def build_nc(dump=False):
    nc = bacc.Bacc(
        "TRN2",
        target_bir_lowering=False,
        debug=False,
        enable_asserts=False,
        num_devices=B,
    )

    x1 = nc.dram_tensor("x1", [C1, S1], BF16, kind="ExternalInput").ap()
    x2 = nc.dram_tensor("x2", [C2, S2], BF16, kind="ExternalInput").ap()
    wk1t = nc.dram_tensor("wk1t", [C1, NH * KD], BF16, kind="ExternalInput").ap()
    wk2t = nc.dram_tensor("wk2t", [C2, NH * KD], BF16, kind="ExternalInput").ap()
    wv2t = nc.dram_tensor("wv2t", [C2, NH * VD], BF16, kind="ExternalInput").ap()
    wot = nc.dram_tensor("wot", [NH * VD, C1], F32R, kind="ExternalInput").ap()
    y = nc.dram_tensor("y", [C1, S1], F32, kind="ExternalOutput").ap()
    dumps = {}
    if dump:
        for nm, shape in (
            ("d_k1t", [P, S1]),
            ("d_v2a", [P, NH * (VD + 1)]),
            ("d_qk0", [P, S1]),
            ("d_expt", [P, S1]),
            ("d_av", [VD + 1, S1]),
            ("d_recip", [1, S1]),
            ("d_oall", [64, S1]),
        ):
            dumps[nm] = nc.dram_tensor(nm, shape, F32, kind="ExternalOutput").ap()

    with tile.TileContext(nc) as tc:
        with (
            tc.tile_pool(name="const", bufs=1) as cpool,
            tc.tile_pool(name="expt", bufs=7) as epool,
            tc.tile_pool(name="norm", bufs=2) as npool,
            tc.tile_pool(name="yout", bufs=2) as ypool,
            tc.tile_pool(name="pmm", bufs=2, space="PSUM") as pmm,
            tc.tile_pool(name="pav", bufs=2, space="PSUM") as pav,
            tc.tile_pool(name="dscr", bufs=2, space="DRAM") as dpool,
        ):
            # ---- load inputs ----
            x1_sb = [cpool.tile([P, S1], BF16, name=f"x1_{c}") for c in range(2)]
            x2_sb = [cpool.tile([P, S2], BF16, name=f"x2_{c}") for c in range(2)]
            wk1t_sb = [cpool.tile([P, 512], BF16, name=f"wk1t_{c}") for c in range(2)]
            wk2t_sb = [cpool.tile([P, 512], BF16, name=f"wk2t_{c}") for c in range(2)]
            wv2t_sb = [cpool.tile([P, 512], BF16, name=f"wv2t_{c}") for c in range(2)]
            wot_sb = [cpool.tile([64, C1], F32R, name=f"wot_{h}") for h in range(NH)]
            for c in range(2):
                nc.sync.dma_start(wk1t_sb[c][:], wk1t[c * P : (c + 1) * P, :])
                nc.sync.dma_start(x1_sb[c][:], x1[c * P : (c + 1) * P, :])
            for c in range(2):
                nc.sync.dma_start(wk2t_sb[c][:], wk2t[c * P : (c + 1) * P, :])
                nc.sync.dma_start(x2_sb[c][:], x2[c * P : (c + 1) * P, :])
            for c in range(2):
                nc.sync.dma_start(wv2t_sb[c][:], wv2t[c * P : (c + 1) * P, :])
            for h in range(NH):
                nc.sync.dma_start(wot_sb[h][:], wot[h * 64 : (h + 1) * 64, :])

            k1t_sb = [cpool.tile([P, S1], BF16, name=f"k1t_{m}") for m in range(4)]
            k2t_sb = [cpool.tile([P, S2], BF16, name=f"k2t_{m}") for m in range(4)]
            # v2 with per-head ones column: [128, head, 65]
            v2a_sb = [
                cpool.tile([P, NH, VD + 1], BF16, name=f"v2a_{s}") for s in range(8)
            ]
            oall_sb = [cpool.tile([64, S1], F32R, name=f"oall_{h}") for h in range(NH)]

            def emit_proj_chunk(pool, wt_sb, xs_sb, kt, m, dump_to=None):
                """kt[m] (bf16 [128, S]) = (wt chunk).T @ xs, both f32r."""
                tag = "qk" if pool is pmm else "pav"
                ps = pool.tile([P, 1024], F32, tag=tag, name=f"pj_{kt[m].name}")
                for nh_ in range(2):
                    for k in range(2):
                        nc.tensor.matmul(
                            ps[:, nh_ * 512 : (nh_ + 1) * 512],
                            wt_sb[k][:, m * P : (m + 1) * P],
                            xs_sb[k][:, nh_ * 512 : (nh_ + 1) * 512],
                            start=(k == 0),
                            stop=(k == 1),
                        )
                nc.vector.tensor_copy(out=kt[m][:], in_=ps[:])
                if dump_to is not None:
                    dt_ = ypool.tile([P, S1], F32, tag="dmp", name="dmp_k")
                    nc.vector.tensor_copy(out=dt_[:], in_=kt[m][:])
                    nc.sync.dma_start(dump_to, dt_[:])

            def emit_v2_pair(sp):
                ps = pav.tile([P, 1024], F32, tag="pav", name=f"pv2_{sp}")
                for half in range(2):
                    s = 2 * sp + half
                    for k in range(2):
                        nc.tensor.matmul(
                            ps[:, half * 512 : (half + 1) * 512],
                            x2_sb[k][:, s * P : (s + 1) * P],
                            wv2t_sb[k][:],
                            start=(k == 0),
                            stop=(k == 1),
                        )
                for half in range(2):
                    s = 2 * sp + half
                    nc.vector.memset(v2a_sb[s][:, :, VD : VD + 1], 1.0)
                    nc.vector.tensor_copy(
                        out=v2a_sb[s][:, :, 0:VD],
                        in_=ps[:, half * 512 : (half + 1) * 512].rearrange(
                            "p (h c) -> p h c", c=VD
                        ),
                    )
                    if dump and s == 0:
                        dt_ = ypool.tile(
                            [P, NH * (VD + 1)], F32, tag="dmp2", name="dv2a"
                        )
                        nc.vector.tensor_copy(
                            out=dt_[:].rearrange("p (h c) -> p h c", c=VD + 1),
                            in_=v2a_sb[0][:],
                        )
                        nc.sync.dma_start(dumps["d_v2a"], dt_[:])

            # ---- prologue: K-chunk 0 projections (attention gates on these) ----
            emit_proj_chunk(
                pmm, wk1t_sb, x1_sb, k1t_sb, 0, dumps.get("d_k1t") if dump else None
            )
            emit_proj_chunk(pmm, wk2t_sb, x2_sb, k2t_sb, 0)

            # ---- attention: pair-packed (a|b) flat pipeline ----
            # qk psum tile [128, 1024] holds head a's q-half in cols 0:512 and
            # head b's in cols 512:1024 (concurrent row-group matmuls); one exp
            # per tile covers both heads.  AV lags one step so PE always has
            # ready work while ACT streams exps.  V2 and the next K-chunk
            # projections ride the pav pool rotation.
            av_tiles = {}
            et_tiles = {}
            pending = []

            def emit_av(c, s2):
                a, b = 2 * c, 2 * c + 1
                if s2 == 0:
                    for h in (a, b):
                        av_tiles[h] = pav.tile(
                            [VD + 1, S1], F32, tag="pav", name=f"av_{h}"
                        )
                for nh_ in range(2):
                    et = et_tiles[(c, s2, nh_)]
                    for idx, h in enumerate((a, b)):
                        nc.tensor.matmul(
                            av_tiles[h][:, nh_ * 512 : (nh_ + 1) * 512],
                            v2a_sb[s2][:, h, :],
                            et[:, idx * 512 : (idx + 1) * 512],
                            start=(s2 == 0),
                            stop=(s2 == 7),
                            skip_group_check=True,
                        )
                for nh_ in range(2):
                    del et_tiles[(c, s2, nh_)]

            def emit_normalize(h):
                avs = npool.tile([VD + 1, S1], F32, tag="avs", name=f"avs_{h}")
                if h >= 6:
                    nc.scalar.copy(out=avs[:], in_=av_tiles[h][:])
                else:
                    nc.vector.tensor_copy(out=avs[:], in_=av_tiles[h][:])
                rcp = npool.tile([VD + 1, S1], F32, tag="rcp", name=f"rcp_{h}")
                nc.vector.reciprocal_approx_fast(rcp[:], avs[:])
                rdram = dpool.tile([S1], F32, tag="rd", name=f"rd_{h}")
                nc.gpsimd.dma_start(rdram[:], rcp[VD : VD + 1, :])
                rep = npool.tile([64, S1], F32, tag="rep", name=f"rep_{h}")
                nc.gpsimd.dma_start(rep[:], rdram[None, :].to_broadcast((64, S1)))
                nc.gpsimd.tensor_mul(out=oall_sb[h][:], in0=avs[0:VD, :], in1=rep[:])
                if dump and h == 0:
                    dt_ = ypool.tile([VD + 1, S1], F32, tag="dmp", name="dav")
                    nc.vector.tensor_copy(out=dt_[:], in_=avs[:])
                    nc.sync.dma_start(dumps["d_av"], dt_[:])
                    nc.sync.dma_start(dumps["d_recip"], rcp[VD : VD + 1, :])
                    dt2 = ypool.tile([64, S1], F32, tag="dmp", name="doall")
                    nc.vector.tensor_copy(out=dt2[:], in_=oall_sb[0][:])
                    nc.sync.dma_start(dumps["d_oall"], dt2[:])

            def flush_av(upto):
                while len(pending) > upto:
                    cc, ss = pending.pop(0)
                    emit_av(cc, ss)
                    if ss == 7:
                        emit_normalize(2 * cc)
                        emit_normalize(2 * cc + 1)
                        if cc + 2 <= 3:
                            emit_proj_chunk(pav, wk1t_sb, x1_sb, k1t_sb, cc + 2)
                            emit_proj_chunk(pav, wk2t_sb, x2_sb, k2t_sb, cc + 2)

            for c in range(4):
                a, b = 2 * c, 2 * c + 1
                for s2 in range(8):
                    qks = []
                    for nh_ in range(2):
                        qk = pmm.tile(
                            [P, S1], F32, tag="qk", name=f"qk_{c}_{s2}_{nh_}"
                        )
                        for idx, h in enumerate((a, b)):
                            ro = (h % 2) * 64
                            nc.tensor.matmul(
                                qk[:, idx * 512 : (idx + 1) * 512],
                                k2t_sb[c][ro : ro + 64, s2 * P : (s2 + 1) * P],
                                k1t_sb[c][ro : ro + 64, nh_ * 512 : (nh_ + 1) * 512],
                                start=True,
                                stop=True,
                            )
                        qks.append(qk)
                    if c == 0:
                        if s2 == 0:
                            emit_v2_pair(0)
                            emit_v2_pair(1)
                        elif s2 == 1:
                            emit_v2_pair(2)
                            emit_v2_pair(3)
                        elif s2 == 2:
                            emit_proj_chunk(pav, wk1t_sb, x1_sb, k1t_sb, 1)
                            emit_proj_chunk(pav, wk2t_sb, x2_sb, k2t_sb, 1)
                    flush_av(2 if c == 0 else 1)
                    for nh_ in range(2):
                        et = epool.tile(
                            [P, S1], BF16, tag="expt", name=f"et_{c}_{s2}_{nh_}"
                        )
                        nc.scalar.activation(
                            et[:],
                            qks[nh_][:],
                            mybir.ActivationFunctionType.Exp,
                            scale=0.125,
                        )
                        et_tiles[(c, s2, nh_)] = et
                        if dump and c == 0 and s2 == 0 and nh_ == 0:
                            dt_ = ypool.tile([P, S1], F32, tag="dmp", name="dqk0")
                            nc.vector.tensor_copy(out=dt_[:], in_=qks[0][:])
                            nc.sync.dma_start(dumps["d_qk0"], dt_[:])
                            dt2 = ypool.tile([P, S1], F32, tag="dmp", name="dexpt")
                            nc.vector.tensor_copy(out=dt2[:], in_=et[:])
                            nc.sync.dma_start(dumps["d_expt"], dt2[:])
                    pending.append((c, s2))
            flush_av(0)

            # ---- final projection: y[mt] = sum_h WoT_h.T @ oall_h ----
            fins = [
                pmm.tile([P, S1], F32, tag="qk", name=f"fin_{mt}") for mt in range(2)
            ]
            for h in range(NH):
                for mt in range(2):
                    for nh_ in range(2):
                        nc.tensor.matmul(
                            fins[mt][:, nh_ * 512 : (nh_ + 1) * 512],
                            wot_sb[h][:, mt * P : (mt + 1) * P],
                            oall_sb[h][:, nh_ * 512 : (nh_ + 1) * 512],
                            start=(h == 0),
                            stop=(h == NH - 1),
                            skip_group_check=True,
                        )
            for mt in range(2):
                for half in range(2):
                    ysb = ypool.tile([P, 512], F32, tag="y", name=f"y_{mt}_{half}")
                    nc.scalar.copy(
                        out=ysb[:], in_=fins[mt][:, half * 512 : (half + 1) * 512]
                    )
                    nc.sync.dma_start(
                        y[mt * P : (mt + 1) * P, half * 512 : (half + 1) * 512],
                        ysb[:],
                    )

    nc.compile()
    return nc


_nc_cache = None


def _get_nc():
    global _nc_cache
    if _nc_cache is None:
        _nc_cache = build_nc()
    return _nc_cache


def _make_in_maps(input1, input2, Wk1, Wk2, Wv2, Wo):
    import ml_dtypes

    bf16 = ml_dtypes.bfloat16
    input1 = np.asarray(input1, dtype=np.float32).astype(bf16)
    input2 = np.asarray(input2, dtype=np.float32).astype(bf16)
    wk1t = np.ascontiguousarray(np.asarray(Wk1, dtype=np.float32).T.astype(bf16))
    wk2t = np.ascontiguousarray(np.asarray(Wk2, dtype=np.float32).T.astype(bf16))
    wv2t = np.ascontiguousarray(np.asarray(Wv2, dtype=np.float32).T.astype(bf16))
    wot = np.ascontiguousarray(np.asarray(Wo, dtype=np.float32).T)
    return [
        {
            "x1": np.ascontiguousarray(input1[b].reshape(C1, S1)),
            "x2": np.ascontiguousarray(input2[b].reshape(C2, S2)),
            "wk1t": wk1t,
            "wk2t": wk2t,
            "wv2t": wv2t,
            "wot": wot,
        }
        for b in range(B)
    ]


def _assemble(results):
    out = np.stack([results[b]["y"] for b in range(B)], axis=0)
    return np.ascontiguousarray(out.reshape(B, C1, 32, 32).astype(np.float32))


def kernel(input1, input2, Wk1, Wk2, Wv2, Wo):
    nc = _get_nc()
    in_maps = _make_in_maps(input1, input2, Wk1, Wk2, Wv2, Wo)
    res = bass_utils.run_bass_kernel_spmd(nc, in_maps, core_ids=list(range(B)))
    return _assemble(res.results)


def kernel_traced(input1, input2, Wk1, Wk2, Wv2, Wo):
    """Like kernel() but with NTFF profiling; returns (out, BassKernelResults)."""
    nc = _get_nc()
    in_maps = _make_in_maps(input1, input2, Wk1, Wk2, Wv2, Wo)
    res = bass_utils.run_bass_kernel_spmd(
        nc, in_maps, core_ids=list(range(B)), trace=True
    )
    return _assemble(res.results), res
